# revision 39
# baseline (speedup 1.0000x reference)
"""AttentionPooling (segment softmax + weighted segment-sum) on 8 trn2 cores.

Strategy: shard nodes equally across cores (no segment alignment needed).
Host pre-builds TWO fp8e3m4 layouts of x: natural [node, D+2ones] in
super-tile order (scatter rhs) and transposed [D, node] in chunk order
(score-MLP rhs) - no on-chip transposes, both PE contractions get their
native layout. Device per 128-node tile, software-pipelined 3 stages deep:
h = tanh(W1^T x^T) (bf16 W1, fp8 x) -> s = hb^T w2 -> e = exp(s) -> A_e
one-hot build (vector, width-scheduled) -> scatter matmul accumulating raw
[rows, D+2] U/Z into two alternating PSUM banks per 124-tile window. The
scatter write width per tile-index comes from a shared (SPMD-identical)
schedule derived from the batch envelope; the first group of each window
writes full 128 rows to clear PSUM. At window end the device adds the two
banks and dumps raw U/Z rows; the host divides U/Z and scatter-adds window
rows into the final [4096, 256] output (straddling segments merge by
addition; never-written rows are masked out). No collectives.

Measured on 8xTRN2 (core-0 profile): ~325 us vs 726 us for the staged
baseline; rel err 1.64e-2 (gate 2e-2), dominated by fp8 quantization of x.
"""

import numpy as np
import ml_dtypes

BF16 = ml_dtypes.bfloat16
XT_NP = ml_dtypes.float8_e3m4  # score-path x dtype (device: float8e3)
XT_BIR = "float8e3"
XN_NP = ml_dtypes.float8_e3m4  # pool-path x dtype
XN_BIR = "float8e3"

# ---------------------------------------------------------------- constants
N_FULL = 1_000_000
D = 256
H = 128
G = 4096
NCORES = 8
P = 128

NC_NODES = N_FULL // NCORES  # 125000 real nodes per core
TILES = 992                  # node tiles per core
NC_PAD = TILES * P           # 126976 padded nodes per core
WINDOWS = 8
TPW = TILES // WINDOWS       # 124 tiles per window
WIN_NODES = TPW * P          # 15872
SUPER = 31                   # xn tiles per DMA super-tile
CHUNK = 32                   # xt tiles per DMA chunk
N_SUPERS = TILES // SUPER    # 32
N_CHUNKS = TILES // CHUNK    # 31
GROUP = 4                    # tiles per h-matmul group
CA = D + 2                   # augmented cols (x | 1 | 1)
OUT_ROWS = WINDOWS * P       # 1024 rows per core
EPS = 1e-30


# ---------------------------------------------------------------- host plan
def _plan(batch):
    """batch: sorted int array [N]. Per-core window bases/rows + rel map."""
    batch = np.asarray(batch).astype(np.int64).ravel()
    n = batch.shape[0]
    assert n == N_FULL
    plans = []
    for c in range(NCORES):
        lo = c * NC_NODES
        hi = lo + NC_NODES
        local = batch[lo:hi]
        rel = np.full(NC_PAD, -1.0, np.float32)
        bases = np.zeros(WINDOWS, np.int64)
        nrows = np.zeros(WINDOWS, np.int64)
        masks = []
        for w in range(WINDOWS):
            a = w * WIN_NODES
            b = min((w + 1) * WIN_NODES, NC_NODES)
            assert a < NC_NODES
            base = int(local[a])
            r = local[a:b] - base
            assert r.min() >= 0 and r.max() < P, (
                f"core {c} window {w}: {P} seg rows exceeded (max rel {r.max()})"
            )
            rel[a:b] = r.astype(np.float32)
            bases[w] = base
            nrows[w] = int(local[b - 1]) - base + 1
            m = np.zeros(int(nrows[w]), bool)
            m[(local[a:b] - base).astype(np.int64)] = True
            masks.append(m)
        plans.append(
            dict(lo=lo, hi=hi, rel=rel, bases=bases, nrows=nrows, masks=masks)
        )

    # shared (SPMD-identical) scatter width schedule per tile-index-in-window:
    # matmul writes rows [0, W_i) with W_i = 32-rounded envelope max; the
    # first group of each window writes full 128 to clear PSUM.
    hi_env = np.full(TPW, 0.0)
    for pl in plans:
        rel_t = pl["rel"].reshape(TILES, P)
        for t in range(TILES):
            i = t % TPW
            valid = rel_t[t][rel_t[t] >= 0]
            if valid.size:
                hi_env[i] = max(hi_env[i], valid.max())
    w_sched = np.full(TPW, P, np.int64)
    for i in range(GROUP, TPW):
        w_sched[i] = min(P, 32 * int(np.ceil((hi_env[i] + 1) / 32)))
        assert hi_env[i] < w_sched[i]
    for pl in plans:
        pl["rel_arr"] = pl["rel"].reshape(TILES, P).T.copy()  # [P, TILES]
        pl["w_sched"] = w_sched
    return plans


def _make_in_maps(x, W1, b1, W2, b2, plans):
    x = np.ascontiguousarray(np.asarray(x), dtype=np.float32)
    W1 = np.asarray(W1, dtype=np.float32).astype(BF16)          # [D, H]
    b1 = np.asarray(b1, dtype=np.float32).reshape(H, 1)
    W2 = np.asarray(W2, dtype=np.float32).reshape(H, 1)
    W2 = np.repeat(W2, 2, axis=1).astype(BF16)                  # [H, 2]
    b2 = np.asarray(b2, dtype=np.float32).reshape(1, 1)
    in_maps = []
    for pl in plans:
        xs = np.zeros((NC_PAD, D), np.float32)
        xs[:NC_NODES] = x[pl["lo"] : pl["hi"]]
        # natural augmented layout, super-tile order:
        # xn[s*128 + p, t*258 + c] = xaug[s*3968 + t*128 + p, c]
        xa = np.ones((NC_PAD, CA), np.float32)
        xa[:, :D] = xs
        xa[NC_NODES:] = 0.0
        xn = (
            xa.reshape(N_SUPERS, SUPER, P, CA)
            .transpose(0, 2, 1, 3)
            .reshape(N_SUPERS * P, SUPER * CA)
            .astype(XN_NP)
        )
        # transposed layout, chunk order:
        # xt[ch*128 + d, k*4096 + j] = xs[ch*4096 + j, k*128 + d]
        xt = (
            xs.reshape(N_CHUNKS, CHUNK * P, 2, P)
            .transpose(0, 3, 2, 1)
            .reshape(N_CHUNKS * P, 2 * CHUNK * P)
            .astype(XT_NP)
        )
        in_maps.append(
            {
                "xn": xn,
                "xt": xt,
                "relseg": pl["rel_arr"],
                "w1": W1,
                "b1": b1,
                "w2": W2,
                "b2": b2,
            }
        )
    return in_maps


def _assemble(outs, plans, dtype):
    U = np.zeros((G, D), np.float64)
    Z = np.zeros((G,), np.float64)
    for pl, o in zip(plans, outs):
        o = np.asarray(o, np.float64)
        for w in range(WINDOWS):
            g0 = int(pl["bases"][w])
            nr = int(pl["nrows"][w])
            m = pl["masks"][w]
            rows = o[w * P : w * P + nr]
            U[g0 : g0 + nr][m] += rows[m][:, :D]
            Z[g0 : g0 + nr][m] += rows[m][:, D]
    y = U / (Z[:, None] + EPS)
    return y.astype(dtype)


# ------------------------------------------------------------ numpy emulator
def _emulate(inputs):
    """Pure-numpy emulation of the device program (for logic validation)."""
    W1 = np.asarray(inputs["W1"], np.float32)
    b1 = np.asarray(inputs["b1"], np.float32)
    b2 = np.asarray(inputs["b2"], np.float32)
    plans = _plan(inputs["batch"])
    in_maps = _make_in_maps(
        inputs["x"], W1, b1, inputs["W2"], b2, plans
    )
    outs = []
    cols = np.arange(P, dtype=np.float32)[None, :]
    for pl, im in zip(plans, in_maps):
        # reconstruct device views from the DMA layouts
        xn = (
            np.asarray(im["xn"], np.float32)
            .reshape(N_SUPERS, P, SUPER, CA)
            .transpose(0, 2, 1, 3)
            .reshape(NC_PAD, CA)
        )
        xt = (
            np.asarray(im["xt"], np.float32)
            .reshape(N_CHUNKS, P, 2, CHUNK * P)
            .transpose(0, 3, 2, 1)
            .reshape(NC_PAD, D)
        )
        w1 = np.asarray(im["w1"], np.float32)
        w2 = np.asarray(im["w2"], np.float32)[:, 0]
        rel = np.asarray(im["relseg"], np.float32).T.reshape(-1)  # node order
        h = np.tanh((xt @ w1) + b1.reshape(1, H)).astype(XT_NP).astype(np.float32)
        s = h @ w2 + float(b2.ravel()[0])
        e = np.exp(s).astype(np.float32)
        out = np.zeros((OUT_ROWS, CA), np.float32)
        for w in range(WINDOWS):
            a, b = w * WIN_NODES, (w + 1) * WIN_NODES
            A = (cols == rel[a:b, None]).astype(np.float32) * e[a:b, None]
            A = A.astype(BF16).astype(np.float32)
            out[w * P : (w + 1) * P] = A.T @ xn[a:b]
        outs.append(out)
    return _assemble(outs, plans, np.float32)


# ------------------------------------------------------------- bass program
_NC_CACHE = {}


def _build_nc(w_sched):
    key = tuple(int(v) for v in w_sched)
    if key in _NC_CACHE:
        return _NC_CACHE[key]
    import concourse.bacc as bacc
    import concourse.mybir as mybir
    import concourse.tile as tile

    f32 = mybir.dt.float32
    bf16 = mybir.dt.bfloat16
    xt_dt = getattr(mybir.dt, XT_BIR)
    AF = mybir.ActivationFunctionType
    ALU = mybir.AluOpType

    nc = bacc.Bacc(None, target_bir_lowering=False)

    xn_dt = getattr(mybir.dt, XN_BIR)
    xn_d = nc.dram_tensor("xn", [N_SUPERS * P, SUPER * CA], xn_dt, kind="ExternalInput")
    xt_d = nc.dram_tensor("xt", [N_CHUNKS * P, 2 * CHUNK * P], xt_dt, kind="ExternalInput")
    rel_d = nc.dram_tensor("relseg", [P, TILES], f32, kind="ExternalInput")
    w1_d = nc.dram_tensor("w1", [D, H], bf16, kind="ExternalInput")
    b1_d = nc.dram_tensor("b1", [H, 1], f32, kind="ExternalInput")
    w2_d = nc.dram_tensor("w2", [H, 2], bf16, kind="ExternalInput")
    b2_d = nc.dram_tensor("b2", [1, 1], f32, kind="ExternalInput")
    out_d = nc.dram_tensor("out", [OUT_ROWS, CA], f32, kind="ExternalOutput")

    with tile.TileContext(nc) as tc:
        with (
            tc.tile_pool(name="singles", bufs=1) as singles,
            tc.tile_pool(name="xn_sup", bufs=3) as xn_pool,
            tc.tile_pool(name="xt_chk", bufs=3) as xt_pool,
            tc.tile_pool(name="hb", bufs=4) as hb_pool,
            tc.tile_pool(name="e", bufs=4) as e_pool,
            tc.tile_pool(name="ae", bufs=20) as ae_pool,
            tc.tile_pool(name="flush", bufs=3) as flush_pool,
            tc.tile_pool(name="ps_h", bufs=2, space="PSUM") as ps_h,
            tc.tile_pool(name="ps_s", bufs=2, space="PSUM") as ps_s,
            tc.tile_pool(name="ps_uz", bufs=2, space="PSUM") as ps_uz,
        ):
            iota_i = singles.tile([P, P], mybir.dt.int32)
            nc.gpsimd.iota(iota_i[:], pattern=[[1, P]], base=0, channel_multiplier=0)
            iota_bf = singles.tile([P, P], bf16)
            nc.vector.tensor_copy(out=iota_bf[:], in_=iota_i[:])

            w1_sb = singles.tile([P, 2, H], bf16)
            w1_r = w1_d[:].rearrange("(k d) m -> k d m", k=2)
            nc.sync.dma_start(out=w1_sb[:, 0, :], in_=w1_r[0])
            nc.sync.dma_start(out=w1_sb[:, 1, :], in_=w1_r[1])
            b1_sb = singles.tile([P, 1], f32)
            nc.sync.dma_start(out=b1_sb[:], in_=b1_d[:])
            w2_sb = singles.tile([P, 2], bf16)
            nc.sync.dma_start(out=w2_sb[:], in_=w2_d[:])
            b2_sb = singles.tile([P, 1], f32)
            nc.sync.dma_start(out=b2_sb[:], in_=b2_d[:].to_broadcast([P, 1]))
            rel_sb = singles.tile([P, TILES], f32)
            nc.sync.dma_start(out=rel_sb[:], in_=rel_d[:])

            sup_cache = {}
            chk_cache = {}

            def get_sup(sg):
                if sg not in sup_cache:
                    t = xn_pool.tile([P, SUPER, CA], xn_dt)
                    nc.sync.dma_start(
                        out=t[:],
                        in_=xn_d[sg * P : (sg + 1) * P, :].rearrange(
                            "p (t c) -> p t c", t=SUPER
                        ),
                    )
                    sup_cache[sg] = t
                return sup_cache[sg]

            def get_chk(cg):
                if cg not in chk_cache:
                    t = xt_pool.tile([P, 2, CHUNK * P], xt_dt)
                    nc.sync.dma_start(
                        out=t[:],
                        in_=xt_d[cg * P : (cg + 1) * P, :].rearrange(
                            "p (k j) -> p k j", k=2
                        ),
                    )
                    chk_cache[cg] = t
                return chk_cache[cg]

            NG = TILES // GROUP  # 248 groups
            GPW = TPW // GROUP   # 31 groups per window
            hb_map = {}
            ae_map = {}
            uz_map = {}

            def stage_a(g):
                # h = tanh(x @ W1 + b1): [hid, 4*nodes] -> hb bf16
                t0 = g * GROUP
                chk = get_chk(t0 // CHUNK)
                off = (t0 % CHUNK) * P
                h_ps = ps_h.tile([P, GROUP, P], f32)
                for k in range(2):
                    nc.tensor.matmul(
                        out=h_ps[:],
                        lhsT=w1_sb[:, k, :],
                        rhs=chk[:, k, off : off + GROUP * P],
                        start=(k == 0),
                        stop=(k == 1),
                    )
                hb = hb_pool.tile([P, GROUP, P], xt_dt)
                nc.scalar.activation(
                    out=hb[:], in_=h_ps[:], func=AF.Tanh, bias=b1_sb[:], scale=1.0
                )
                hb_map[g] = hb

            def stage_b(g):
                # s per tile -> e = exp(s + b2) -> A_e one-hot builds
                hb = hb_map.pop(g)
                s_ps = ps_s.tile([P, GROUP, 2], f32)
                for t in range(GROUP):
                    nc.tensor.matmul(
                        out=s_ps[:, t, :],
                        lhsT=hb[:, t, :],
                        rhs=w2_sb[:],
                        start=True,
                        stop=True,
                    )
                e_sb = e_pool.tile([P, GROUP], f32)
                nc.scalar.activation(
                    out=e_sb[:],
                    in_=s_ps[:, :, 0],
                    func=AF.Exp,
                    bias=b2_sb[:],
                    scale=1.0,
                )
                aes = []
                for t in range(GROUP):
                    gt = g * GROUP + t
                    W = int(w_sched[gt % TPW])
                    ae = ae_pool.tile([P, W], bf16, name=f"ae_{W}")
                    nc.vector.tensor_scalar(
                        out=ae[:],
                        in0=iota_bf[:, :W],
                        scalar1=rel_sb[:, gt : gt + 1],
                        scalar2=e_sb[:, t : t + 1],
                        op0=ALU.is_equal,
                        op1=ALU.mult,
                    )
                    aes.append((ae, W))
                ae_map[g] = aes

            def stage_c(g):
                # scatter matmuls accumulating window U/Z into two alternating
                # PSUM banks (removes the accumulate RAW chain); flush adds them
                w = g // GPW
                if g % GPW == 0:
                    uz_map[w] = (
                        ps_uz.tile([P, CA], f32, name="uz_a"),
                        ps_uz.tile([P, CA], f32, name="uz_b"),
                    )
                uz_pair = uz_map[w]
                aes = ae_map.pop(g)
                for t in range(GROUP):
                    gt = g * GROUP + t
                    i = gt % TPW
                    sup = get_sup(gt // SUPER)
                    ae, rows = aes[t]
                    nc.tensor.matmul(
                        out=uz_pair[i % 2][0:rows, :],
                        lhsT=ae[:],
                        rhs=sup[:, gt % SUPER, :],
                        start=(i < 2),
                        stop=(i >= TPW - 2),
                        skip_group_check=True,
                    )
                if g % GPW == GPW - 1:
                    uz_a, uz_b = uz_map.pop(w)
                    uz_sb = flush_pool.tile([P, CA], f32)
                    nc.vector.tensor_copy(out=uz_sb[:], in_=uz_a[:])
                    nc.vector.tensor_add(
                        out=uz_sb[:], in0=uz_sb[:], in1=uz_b[:]
                    )
                    nc.sync.dma_start(
                        out=out_d[w * P : (w + 1) * P, :], in_=uz_sb[:]
                    )

            for i in range(NG + 2):
                if i < NG:
                    stage_a(i)
                if 1 <= i <= NG:
                    stage_b(i - 1)
                if i >= 2:
                    stage_c(i - 2)

    nc.finalize()
    _NC_CACHE[key] = nc
    return nc


def _run(inputs, trace=False):
    from concourse.bass_utils import run_bass_kernel_spmd

    plans = _plan(inputs["batch"])
    in_maps = _make_in_maps(
        inputs["x"], inputs["W1"], inputs["b1"], inputs["W2"], inputs["b2"], plans
    )
    nc = _build_nc(plans[0]["w_sched"])
    res = run_bass_kernel_spmd(
        nc, in_maps, core_ids=list(range(NCORES)), trace=trace
    )
    outs = [r["out"] for r in res.results]
    final = _assemble(outs, plans, np.float32)
    return final, res


def kernel(**inputs):
    return _run(inputs, trace=False)[0]
